# revision 14
# baseline (speedup 1.0000x reference)
"""ChebConv (K=3) GNN message passing on 8 Trainium2 NeuronCores.

Strategy (1D node partition, per sharding hint):
  - Nodes padded to NPAD rows and split into 8 contiguous blocks; core c owns
    dst rows [c*ROWS, (c+1)*ROWS) = WPC windows of 128 dst nodes each.
  - Both D^-1/2 scalings are folded into per-edge weights
    w_e = dinv[src] * dinv[dst], so each unnormalized-Laplacian application is
    a pure weighted segment-sum: h[dst] = sum_e w_e * x[src_e].
  - On device, each window's edges are processed as tiles of 128 edge slots:
    dma_gather fetches x[src] rows ([128 slots, 64] per tile), one fused DVE
    tensor_scalar builds the weighted one-hot lhsT ([slot, dst_local] =
    w * (iota == dstl)), and the tensor engine accumulates the window's
    segment-sum in PSUM across the window's tiles.
  - Chebyshev recurrence combines are node-local (ACT/DVE); X1 shards are
    exchanged between rounds via an on-device AllGather.
  - Edge slots are split into low/high src halves (gather indices are int16)
    and padded to uniform per-window tile counts (GL low + GH high) so the
    same NEFF runs on all 8 cores; per-core variation lives in input tables.
"""

import sys

for _p in ("/opt/trn_rl_repo",):
    if _p not in sys.path:
        sys.path.insert(0, _p)

import numpy as np

# Problem shape (hardcoded per contract).
N, E, D = 60000, 1200000, 64
NCORES = 8
WIN = 128           # dst nodes per window (PSUM partition dim)
WPC = 60            # windows per core
ROWS = WPC * WIN    # 7680 dst rows per core
NPAD = NCORES * ROWS  # 61440 padded node rows
SPLIT = 32768       # int16 gather index range per source half
CW = 4              # windows per gather chunk (WPC % CW == 0)
BF16 = True         # bf16 gather rows + one-hot (4x DVE mode, PE FWL)


def _preprocess(src, dst, w_e):
    """Build per-core gather/one-hot tables.

    Returns (GL, GH, tables) where tables[c] = dict(idx16, dstl, wts).
    """
    src = np.asarray(src, dtype=np.int64)
    dst = np.asarray(dst, dtype=np.int64)
    w_e = np.asarray(w_e, dtype=np.float32)

    gwin = dst // WIN          # global window id
    core = gwin // WPC
    wloc = gwin % WPC
    low = src < SPLIT

    # Per (core, window, half) tile counts -> uniform GL / GH.
    nlo = np.zeros((NCORES, WPC), np.int64)
    nhi = np.zeros((NCORES, WPC), np.int64)
    np.add.at(nlo, (core[low], wloc[low]), 1)
    np.add.at(nhi, (core[~low], wloc[~low]), 1)
    GL = int(np.max(np.ceil(nlo / 128)))
    GH = int(np.max(np.ceil(nhi / 128)))
    G = GL + GH
    TILES = WPC * G
    SLOTS = TILES * 128
    HI0 = WPC * GL * 128  # high-stream slot offset

    # Sort edges by (core, window, high-flag) once; then slice runs.
    key = (core * WPC + wloc) * 2 + (~low)
    order = np.argsort(key, kind="stable")
    s_src = src[order]
    s_w = w_e[order]
    s_dstl = (dst[order] % WIN).astype(np.float32)
    s_low = low[order]

    # Start offset of each (core, window, half) run in the sorted arrays.
    counts = np.zeros(NCORES * WPC * 2, np.int64)
    np.add.at(counts, key, 1)
    starts = np.concatenate([[0], np.cumsum(counts)])

    tables = []
    for c in range(NCORES):
        idx_slot = np.zeros(SLOTS, np.int16)
        w_slot = np.zeros(SLOTS, np.float32)
        dstl_slot = np.zeros(SLOTS, np.float32)
        for w in range(WPC):
            kbase = (c * WPC + w) * 2
            # low half
            a, b = starts[kbase], starts[kbase + 1]
            n = b - a
            o = w * GL * 128
            if n:
                assert s_low[a:b].all()
                idx_slot[o:o + n] = s_src[a:b].astype(np.int16)
                w_slot[o:o + n] = s_w[a:b]
                dstl_slot[o:o + n] = s_dstl[a:b]
            # high half
            a, b = starts[kbase + 1], starts[kbase + 2]
            n = b - a
            o = HI0 + w * GH * 128
            if n:
                assert not s_low[a:b].any()
                idx_slot[o:o + n] = (s_src[a:b] - SPLIT).astype(np.int16)
                w_slot[o:o + n] = s_w[a:b]
                dstl_slot[o:o + n] = s_dstl[a:b]

        # idx16 layout: stream pos i -> [i % 16, i // 16], replicated to
        # all 8 groups of 16 partitions (one per GPSIMD Q7 core).
        idx16 = np.tile(idx_slot.reshape(SLOTS // 16, 16).T, (8, 1))
        # Per-tile tables in STREAM tile order: [partition(slot%128), tile].
        dstl = dstl_slot.reshape(TILES, 128).T.copy()
        wts = w_slot.reshape(TILES, 128).T.copy()
        if BF16:
            import ml_dtypes
            dstl = dstl.astype(ml_dtypes.bfloat16)
            wts = wts.astype(ml_dtypes.bfloat16)
        tables.append({"idx16": idx16, "dstl": dstl, "wts": wts})

    return GL, GH, tables


def _build_bass(GL, GH, re_norm):
    import concourse.bass as bass
    import concourse.bacc as bacc
    import concourse.mybir as mybir
    import concourse.tile as tile
    from contextlib import ExitStack

    f32 = mybir.dt.float32
    i16 = mybir.dt.int16
    bf16 = mybir.dt.bfloat16
    gdt = bf16 if BF16 else f32      # gather-row / one-hot dtype
    GC = 2 * D if BF16 else D        # gather row width (256B either way)
    AF = mybir.ActivationFunctionType
    OP = mybir.AluOpType

    G = GL + GH
    TILES = WPC * G
    SLOTS = TILES * 128
    HI0 = WPC * GL * 128

    a1 = float(-re_norm)            # X1 = a1*h1 + b1*X0
    b1 = float(re_norm - 1.0)
    a2 = float(-2.0 * re_norm)      # X2 = a2*h2 + b2*X1 - X0
    b2 = float(2.0 * (re_norm - 1.0))

    nc = bacc.Bacc(
        "TRN2",
        target_bir_lowering=False,
        debug=False,
        enable_asserts=False,
        num_devices=NCORES,
    )
    xg1 = nc.dram_tensor("xg1", [NPAD, GC], gdt, kind="ExternalInput")
    x0own = nc.dram_tensor("x0own", [ROWS, D], f32, kind="ExternalInput")
    idx16_d = nc.dram_tensor("idx16", [128, SLOTS // 16], i16, kind="ExternalInput")
    dstl_d = nc.dram_tensor("dstl", [128, TILES], gdt, kind="ExternalInput")
    wts_d = nc.dram_tensor("wts", [128, TILES], gdt, kind="ExternalInput")
    out_d = nc.dram_tensor("out", [ROWS, 3 * D], f32, kind="ExternalOutput")
    import ml_dtypes
    _iota_np = np.broadcast_to(np.arange(128), (128, 128))
    iota_d = nc.inline_tensor(
        _iota_np.astype(ml_dtypes.bfloat16 if BF16 else np.float32),
        name="iota",
    )

    with ExitStack() as ctx:
        tc = ctx.enter_context(tile.TileContext(nc))
        dram = ctx.enter_context(tc.tile_pool(name="dram", bufs=1, space="DRAM"))
        x1shard = dram.tile([ROWS, GC], gdt)
        x1full = dram.tile([NPAD, GC], gdt, addr_space="Shared")

        cpool = ctx.enter_context(tc.tile_pool(name="const", bufs=1))
        idx_sb = cpool.tile([128, SLOTS // 16], i16)
        nc.sync.dma_start(out=idx_sb[:], in_=idx16_d[:])
        dstl_sb = cpool.tile([128, TILES], gdt)
        nc.sync.dma_start(out=dstl_sb[:], in_=dstl_d[:])
        wts_sb = cpool.tile([128, TILES], gdt)
        nc.sync.dma_start(out=wts_sb[:], in_=wts_d[:])
        iota_sb = cpool.tile([128, 128], gdt)
        nc.sync.dma_start(out=iota_sb[:], in_=iota_d[:])
        x0_sb = cpool.tile([128, WPC * D], f32)
        nc.sync.dma_start(
            out=x0_sb[:].rearrange("p (w d) -> p w d", d=D),
            in_=x0own[:].rearrange("(w p) d -> p w d", p=128),
        )
        x1_sb = cpool.tile([128, WPC * D], f32)

        gpool = ctx.enter_context(tc.tile_pool(name="gath", bufs=2))
        ohpool = ctx.enter_context(tc.tile_pool(name="oh", bufs=8))
        pspool = ctx.enter_context(tc.tile_pool(name="ps", bufs=4, space="PSUM"))
        mpool = ctx.enter_context(tc.tile_pool(name="misc", bufs=4))
        opool = ctx.enter_context(tc.tile_pool(name="outs", bufs=3))

        L_low = CW * GL * 128
        L_high = CW * GH * 128
        reg_low = nc.gpsimd.alloc_register("n_idx_low")
        nc.gpsimd.reg_mov(reg_low, L_low)
        if L_high != L_low:
            reg_high = nc.gpsimd.alloc_register("n_idx_high")
            nc.gpsimd.reg_mov(reg_high, L_high)
        else:
            reg_high = reg_low

        def do_round(xsrc, second):
            for wlo in range(0, WPC, CW):
                glow = gpool.tile([128, CW * GL, GC], gdt, tag="glow")
                ghigh = gpool.tile([128, CW * GH, GC], gdt, tag="ghigh")
                s0 = wlo * GL * 128
                L = CW * GL * 128
                nc.gpsimd.dma_gather(
                    out_ap=glow[:, :, :],
                    in_ap=xsrc[0:SPLIT, :],
                    idxs_ap=idx_sb[:, s0 // 16:(s0 + L) // 16],
                    num_idxs=L,
                    num_idxs_reg=reg_low,
                    elem_size=GC,
                    single_packet=False,
                )
                s0h = HI0 + wlo * GH * 128
                Lh = CW * GH * 128
                nc.gpsimd.dma_gather(
                    out_ap=ghigh[:, :, :],
                    in_ap=xsrc[SPLIT:NPAD, :],
                    idxs_ap=idx_sb[:, s0h // 16:(s0h + Lh) // 16],
                    num_idxs=Lh,
                    num_idxs_reg=reg_high,
                    elem_size=GC,
                    single_packet=False,
                )
                if second:
                    outc = opool.tile([128, CW * 3 * D], f32, tag="outc")
                else:
                    x1bf = opool.tile([128, CW * GC], gdt, tag="x1bf") if BF16 else None
                for wi in range(CW):
                    w = wlo + wi
                    ps = pspool.tile([128, D], f32)
                    for t in range(G):
                        if t < GL:
                            g_ap = glow[:, wi * GL + t, 0:D]
                            sti = w * GL + t
                        else:
                            g_ap = ghigh[:, wi * GH + (t - GL), 0:D]
                            sti = WPC * GL + w * GH + (t - GL)
                        oh = ohpool.tile([128, 128], gdt, tag="oh")
                        nc.vector.tensor_scalar(
                            out=oh[:],
                            in0=iota_sb[:],
                            scalar1=dstl_sb[:, sti:sti + 1],
                            scalar2=wts_sb[:, sti:sti + 1],
                            op0=OP.is_equal,
                            op1=OP.mult,
                        )
                        nc.tensor.matmul(
                            ps[:],
                            lhsT=oh[:],
                            rhs=g_ap,
                            start=(t == 0),
                            stop=(t == G - 1),
                        )
                    x0w = x0_sb[:, w * D:(w + 1) * D]
                    x1w = x1_sb[:, w * D:(w + 1) * D]
                    if not second:
                        # X1 = a1*h + b1*X0
                        tmp = mpool.tile([128, D], f32, tag="t1")
                        nc.scalar.activation(tmp[:], ps[:], AF.Copy, scale=a1)
                        if b1 == 1.0:
                            nc.vector.tensor_tensor(
                                out=x1w, in0=tmp[:], in1=x0w, op=OP.add
                            )
                        else:
                            xb = mpool.tile([128, D], f32, tag="t2")
                            nc.scalar.activation(xb[:], x0w, AF.Copy, scale=b1)
                            nc.vector.tensor_tensor(
                                out=x1w, in0=tmp[:], in1=xb[:], op=OP.add
                            )
                        if BF16:
                            nc.vector.tensor_copy(
                                out=x1bf[:, wi * GC:wi * GC + D], in_=x1w
                            )
                    else:
                        # X2 = a2*h + b2*X1 - X0
                        tmp = mpool.tile([128, D], f32, tag="t1")
                        nc.scalar.activation(tmp[:], ps[:], AF.Copy, scale=a2)
                        xb = mpool.tile([128, D], f32, tag="t2")
                        nc.scalar.activation(xb[:], x1w, AF.Copy, scale=b2)
                        t3 = mpool.tile([128, D], f32, tag="t3")
                        nc.vector.tensor_tensor(
                            out=t3[:], in0=tmp[:], in1=xb[:], op=OP.add
                        )
                        x2 = mpool.tile([128, D], f32, tag="t4")
                        nc.vector.tensor_tensor(
                            out=x2[:], in0=t3[:], in1=x0w, op=OP.subtract
                        )
                        # relu into the chunk output staging tile
                        ob = wi * 3 * D
                        nc.scalar.activation(outc[:, ob:ob + D], x0w, AF.Relu)
                        nc.scalar.activation(outc[:, ob + D:ob + 2 * D], x1w, AF.Relu)
                        nc.scalar.activation(outc[:, ob + 2 * D:ob + 3 * D], x2[:], AF.Relu)
                if not second:
                    # ship this chunk's X1 windows to the DRAM shard
                    if BF16:
                        nc.sync.dma_start(
                            out=x1shard[wlo * 128:(wlo + CW) * 128, :].rearrange(
                                "(w p) d -> p w d", p=128
                            ),
                            in_=x1bf[:].rearrange("p (w d) -> p w d", d=GC),
                        )
                    else:
                        nc.sync.dma_start(
                            out=x1shard[wlo * 128:(wlo + CW) * 128, :].rearrange(
                                "(w p) d -> p w d", p=128
                            ),
                            in_=x1_sb[:, wlo * D:(wlo + CW) * D].rearrange(
                                "p (w d) -> p w d", d=D
                            ),
                        )
                else:
                    nc.sync.dma_start(
                        out=out_d[wlo * 128:(wlo + CW) * 128, :].rearrange(
                            "(w p) d -> p w d", p=128
                        ),
                        in_=outc[:].rearrange("p (w d) -> p w d", d=3 * D),
                    )

        do_round(xg1, False)
        nc.gpsimd.collective_compute(
            "AllGather",
            mybir.AluOpType.bypass,
            replica_groups=[list(range(NCORES))],
            ins=[x1shard[:]],
            outs=[x1full[:]],
        )
        do_round(x1full, True)

    nc.finalize()
    return nc


def _make_in_maps(feat, src, dst, w_e):
    GL, GH, tables = _preprocess(src, dst, w_e)
    xpad = np.zeros((NPAD, D), np.float32)
    xpad[:N] = np.asarray(feat, np.float32)
    in_maps = []
    for c in range(NCORES):
        t = tables[c]
        in_maps.append(
            {
                "xg1": xpad,
                "x0own": xpad[c * ROWS:(c + 1) * ROWS],
                "idx16": t["idx16"],
                "dstl": t["dstl"],
                "wts": t["wts"],
            }
        )
    return GL, GH, in_maps


_CACHE = {}


def _get_program(feat, src, dst, lambda_max):
    re_norm = float(np.float32(2.0) / np.asarray(lambda_max, np.float32).reshape(-1)[0])
    deg = np.bincount(np.asarray(dst, np.int64), minlength=N).astype(np.float32)
    d_inv = np.maximum(deg, 1.0) ** -0.5
    src64 = np.asarray(src, np.int64)
    dst64 = np.asarray(dst, np.int64)
    w_e = (d_inv[src64] * d_inv[dst64]).astype(np.float32)
    GL, GH, in_maps = _make_in_maps(feat, src, dst, w_e)
    key = (GL, GH, re_norm)
    if key not in _CACHE:
        _CACHE[key] = _build_bass(GL, GH, re_norm)
    return _CACHE[key], in_maps


def kernel(feat, src, dst, lambda_max):
    from concourse.bass_utils import run_bass_kernel_spmd

    nc, in_maps = _get_program(feat, src, dst, lambda_max)
    res = run_bass_kernel_spmd(nc, in_maps, core_ids=list(range(NCORES)))
    kernel.last_exec_time_ns = res.exec_time_ns
    out = np.concatenate([res.results[c]["out"] for c in range(NCORES)], axis=0)
    return np.ascontiguousarray(out[:N])


# revision 20
# speedup vs baseline: 1.0187x; 1.0187x over previous
"""ChebConv (K=3) GNN message passing on 8 Trainium2 NeuronCores.

Strategy (1D node partition, per sharding hint):
  - Nodes padded to NPAD rows and split into 8 contiguous blocks; core c owns
    dst rows [c*ROWS, (c+1)*ROWS) = WPC windows of 128 dst nodes each.
  - Both D^-1/2 scalings are folded into per-edge weights
    w_e = dinv[src] * dinv[dst], so each unnormalized-Laplacian application is
    a pure weighted segment-sum: h[dst] = sum_e w_e * x[src_e].
  - On device, each window's edges are processed as tiles of 128 edge slots:
    dma_gather fetches x[src] rows ([128 slots, 64] per tile), one fused DVE
    tensor_scalar builds the weighted one-hot lhsT ([slot, dst_local] =
    w * (iota == dstl)), and the tensor engine accumulates the window's
    segment-sum in PSUM across the window's tiles.
  - Chebyshev recurrence combines are node-local (ACT/DVE); X1 shards are
    exchanged between rounds via an on-device AllGather.
  - Edge slots are split into low/high src halves (gather indices are int16)
    and padded to uniform per-window tile counts (GL low + GH high) so the
    same NEFF runs on all 8 cores; per-core variation lives in input tables.
"""

import sys

for _p in ("/opt/trn_rl_repo",):
    if _p not in sys.path:
        sys.path.insert(0, _p)

import numpy as np

# Problem shape (hardcoded per contract).
N, E, D = 60000, 1200000, 64
NCORES = 8
WIN = 128           # dst nodes per window (PSUM partition dim)
WPC = 60            # windows per core
ROWS = WPC * WIN    # 7680 dst rows per core
NPAD = NCORES * ROWS  # 61440 padded node rows
SPLIT = 32768       # int16 gather index range per source half
CW = 4              # windows per gather chunk (WPC % CW == 0)
BF16 = True         # bf16 gather rows + one-hot (4x DVE mode, PE FWL)


def _preprocess(src, dst, w_e):
    """Build per-core gather/one-hot tables.

    Returns (GL, GH, tables) where tables[c] = dict(idx16, dstl, wts).
    """
    src = np.asarray(src, dtype=np.int64)
    dst = np.asarray(dst, dtype=np.int64)
    w_e = np.asarray(w_e, dtype=np.float32)

    gwin = dst // WIN          # global window id
    core = gwin // WPC
    wloc = gwin % WPC
    low = src < SPLIT

    # Per (core, window, half) tile counts -> uniform GL / GH.
    nlo = np.zeros((NCORES, WPC), np.int64)
    nhi = np.zeros((NCORES, WPC), np.int64)
    np.add.at(nlo, (core[low], wloc[low]), 1)
    np.add.at(nhi, (core[~low], wloc[~low]), 1)
    GL = int(np.max(np.ceil(nlo / 128)))
    GH = int(np.max(np.ceil(nhi / 128)))
    G = GL + GH
    TILES = WPC * G
    SLOTS = TILES * 128
    HI0 = WPC * GL * 128  # high-stream slot offset

    # Sort edges by (core, window, high-flag) once; then slice runs.
    key = (core * WPC + wloc) * 2 + (~low)
    order = np.argsort(key, kind="stable")
    s_src = src[order]
    s_w = w_e[order]
    s_dstl = (dst[order] % WIN).astype(np.float32)
    s_low = low[order]

    # Start offset of each (core, window, half) run in the sorted arrays.
    counts = np.zeros(NCORES * WPC * 2, np.int64)
    np.add.at(counts, key, 1)
    starts = np.concatenate([[0], np.cumsum(counts)])

    tables = []
    for c in range(NCORES):
        idx_slot = np.zeros(SLOTS, np.int16)
        w_slot = np.zeros(SLOTS, np.float32)
        dstl_slot = np.zeros(SLOTS, np.float32)
        for w in range(WPC):
            kbase = (c * WPC + w) * 2
            # low half
            a, b = starts[kbase], starts[kbase + 1]
            n = b - a
            o = w * GL * 128
            if n:
                assert s_low[a:b].all()
                idx_slot[o:o + n] = s_src[a:b].astype(np.int16)
                w_slot[o:o + n] = s_w[a:b]
                dstl_slot[o:o + n] = s_dstl[a:b]
            # high half
            a, b = starts[kbase + 1], starts[kbase + 2]
            n = b - a
            o = HI0 + w * GH * 128
            if n:
                assert not s_low[a:b].any()
                idx_slot[o:o + n] = (s_src[a:b] - SPLIT).astype(np.int16)
                w_slot[o:o + n] = s_w[a:b]
                dstl_slot[o:o + n] = s_dstl[a:b]

        # idx16 layout: stream pos i -> [i % 16, i // 16], replicated to
        # all 8 groups of 16 partitions (one per GPSIMD Q7 core).
        idx16 = np.tile(idx_slot.reshape(SLOTS // 16, 16).T, (8, 1))
        # Per-tile tables in STREAM tile order: [partition(slot%128), tile].
        dstl = dstl_slot.reshape(TILES, 128).T.copy()
        wts = w_slot.reshape(TILES, 128).T.copy()
        tables.append({"idx16": idx16, "dstl": dstl, "wts": wts})

    return GL, GH, tables


def _build_bass(GL, GH, re_norm):
    import concourse.bass as bass
    import concourse.bacc as bacc
    import concourse.mybir as mybir
    import concourse.tile as tile
    from contextlib import ExitStack

    f32 = mybir.dt.float32
    i16 = mybir.dt.int16
    bf16 = mybir.dt.bfloat16
    gdt = bf16 if BF16 else f32      # gather-row / one-hot dtype
    GC = 2 * D if BF16 else D        # gather row width (256B either way)
    AF = mybir.ActivationFunctionType
    OP = mybir.AluOpType

    G = GL + GH
    TILES = WPC * G
    SLOTS = TILES * 128
    HI0 = WPC * GL * 128

    a1 = float(-re_norm)            # X1 = a1*h1 + b1*X0
    b1 = float(re_norm - 1.0)
    a2 = float(-2.0 * re_norm)      # X2 = a2*h2 + b2*X1 - X0
    b2 = float(2.0 * (re_norm - 1.0))

    nc = bacc.Bacc(
        "TRN2",
        target_bir_lowering=False,
        debug=False,
        enable_asserts=False,
        num_devices=NCORES,
        num_swdge_queues=2,
    )
    xg1 = nc.dram_tensor("xg1", [NPAD, GC], gdt, kind="ExternalInput")
    x0own = nc.dram_tensor("x0own", [ROWS, D], f32, kind="ExternalInput")
    idx16_d = nc.dram_tensor("idx16", [128, SLOTS // 16], i16, kind="ExternalInput")
    dstl_d = nc.dram_tensor("dstl", [128, TILES], f32, kind="ExternalInput")
    wts_d = nc.dram_tensor("wts", [128, TILES], f32, kind="ExternalInput")
    out_d = nc.dram_tensor("out", [ROWS, 3 * D], f32, kind="ExternalOutput")
    import ml_dtypes
    _iota_np = np.broadcast_to(np.arange(128), (128, 128))
    iota_d = nc.inline_tensor(
        _iota_np.astype(ml_dtypes.bfloat16 if BF16 else np.float32),
        name="iota",
    )

    with ExitStack() as ctx:
        tc = ctx.enter_context(tile.TileContext(nc))
        dram = ctx.enter_context(tc.tile_pool(name="dram", bufs=1, space="DRAM"))
        x1shard = dram.tile([ROWS, GC], gdt)
        x1full = dram.tile([NPAD, GC], gdt, addr_space="Shared")

        cpool = ctx.enter_context(tc.tile_pool(name="const", bufs=1))
        idx_sb = cpool.tile([128, SLOTS // 16], i16)
        nc.sync.dma_start(out=idx_sb[:], in_=idx16_d[:])
        dstl_sb = cpool.tile([128, TILES], f32)
        nc.sync.dma_start(out=dstl_sb[:], in_=dstl_d[:])
        wts_sb = cpool.tile([128, TILES], f32)
        nc.sync.dma_start(out=wts_sb[:], in_=wts_d[:])
        iota_sb = cpool.tile([128, 128], gdt)
        nc.sync.dma_start(out=iota_sb[:], in_=iota_d[:])
        x0_sb = cpool.tile([128, WPC * D], f32)
        nc.sync.dma_start(
            out=x0_sb[:].rearrange("p (w d) -> p w d", d=D),
            in_=x0own[:].rearrange("(w p) d -> p w d", p=128),
        )
        x1_sb = cpool.tile([128, WPC * D], f32)

        gpool = ctx.enter_context(tc.tile_pool(name="gath", bufs=2))
        ohpool = ctx.enter_context(tc.tile_pool(name="oh", bufs=8))
        pspool = ctx.enter_context(tc.tile_pool(name="ps", bufs=8, space="PSUM"))
        mpool = ctx.enter_context(tc.tile_pool(name="misc", bufs=4))
        opool = ctx.enter_context(tc.tile_pool(name="outs", bufs=3))

        L_low = CW * GL * 128
        L_high = CW * GH * 128
        reg_low = nc.gpsimd.alloc_register("n_idx_low")
        nc.gpsimd.reg_mov(reg_low, L_low)
        if L_high != L_low:
            reg_high = nc.gpsimd.alloc_register("n_idx_high")
            nc.gpsimd.reg_mov(reg_high, L_high)
        else:
            reg_high = reg_low

        def do_round(xsrc, second):
            for wlo in range(0, WPC, CW):
                glow = gpool.tile([128, CW * GL, GC], gdt, tag="glow")
                ghigh = gpool.tile([128, CW * GH, GC], gdt, tag="ghigh")
                s0 = wlo * GL * 128
                L = CW * GL * 128
                nc.gpsimd.dma_gather(
                    out_ap=glow[:, :, :],
                    in_ap=xsrc[0:SPLIT, :],
                    idxs_ap=idx_sb[:, s0 // 16:(s0 + L) // 16],
                    num_idxs=L,
                    num_idxs_reg=reg_low,
                    elem_size=GC,
                    single_packet=False,
                )
                s0h = HI0 + wlo * GH * 128
                Lh = CW * GH * 128
                nc.gpsimd.dma_gather(
                    out_ap=ghigh[:, :, :],
                    in_ap=xsrc[SPLIT:NPAD, :],
                    idxs_ap=idx_sb[:, s0h // 16:(s0h + Lh) // 16],
                    num_idxs=Lh,
                    num_idxs_reg=reg_high,
                    elem_size=GC,
                    single_packet=False,
                    queue_num=1,
                )
                if second:
                    outc = opool.tile([128, CW * 3 * D], f32, tag="outc")
                else:
                    x1bf = None
                    if BF16:
                        x1bf = opool.tile([128, CW * GC], gdt, tag="x1bf",
                                          name="x1bf")
                        nc.gpsimd.memset(x1bf[:], 0)
                for wi in range(CW):
                    w = wlo + wi
                    ps = pspool.tile([128, D], f32)
                    for t in range(G):
                        if t < GL:
                            g_ap = glow[:, wi * GL + t, 0:D]
                            sti = w * GL + t
                        else:
                            g_ap = ghigh[:, wi * GH + (t - GL), 0:D]
                            sti = WPC * GL + w * GH + (t - GL)
                        oh = ohpool.tile([128, 128], gdt, tag="oh")
                        nc.vector.tensor_scalar(
                            out=oh[:],
                            in0=iota_sb[:],
                            scalar1=dstl_sb[:, sti:sti + 1],
                            scalar2=wts_sb[:, sti:sti + 1],
                            op0=OP.is_equal,
                            op1=OP.mult,
                        )
                        nc.tensor.matmul(
                            ps[:],
                            lhsT=oh[:],
                            rhs=g_ap,
                            start=(t == 0),
                            stop=(t == G - 1),
                        )
                    x0w = x0_sb[:, w * D:(w + 1) * D]
                    x1w = x1_sb[:, w * D:(w + 1) * D]
                    if not second:
                        # X1 = a1*h + b1*X0
                        tmp = mpool.tile([128, D], f32, tag="t1")
                        nc.scalar.activation(tmp[:], ps[:], AF.Copy, scale=a1)
                        if b1 == 1.0:
                            nc.vector.tensor_tensor(
                                out=x1w, in0=tmp[:], in1=x0w, op=OP.add
                            )
                        else:
                            xb = mpool.tile([128, D], f32, tag="t2")
                            nc.scalar.activation(xb[:], x0w, AF.Copy, scale=b1)
                            nc.vector.tensor_tensor(
                                out=x1w, in0=tmp[:], in1=xb[:], op=OP.add
                            )
                        if BF16:
                            nc.vector.tensor_copy(
                                out=x1bf[:, wi * GC:wi * GC + D], in_=x1w
                            )
                    else:
                        # X2 = a2*h + b2*X1 - X0
                        tmp = mpool.tile([128, D], f32, tag="t1")
                        nc.scalar.activation(tmp[:], ps[:], AF.Copy, scale=a2)
                        xb = mpool.tile([128, D], f32, tag="t2")
                        nc.scalar.activation(xb[:], x1w, AF.Copy, scale=b2)
                        t3 = mpool.tile([128, D], f32, tag="t3")
                        nc.vector.tensor_tensor(
                            out=t3[:], in0=tmp[:], in1=xb[:], op=OP.add
                        )
                        x2 = mpool.tile([128, D], f32, tag="t4")
                        nc.vector.tensor_tensor(
                            out=x2[:], in0=t3[:], in1=x0w, op=OP.subtract
                        )
                        # relu into the chunk output staging tile
                        ob = wi * 3 * D
                        nc.scalar.activation(outc[:, ob:ob + D], x0w, AF.Relu)
                        nc.scalar.activation(outc[:, ob + D:ob + 2 * D], x1w, AF.Relu)
                        nc.scalar.activation(outc[:, ob + 2 * D:ob + 3 * D], x2[:], AF.Relu)
                if not second:
                    # ship this chunk's X1 windows to the DRAM shard
                    if BF16:
                        nc.sync.dma_start(
                            out=x1shard[wlo * 128:(wlo + CW) * 128, :].rearrange(
                                "(w p) d -> p w d", p=128
                            ),
                            in_=x1bf[:].rearrange("p (w d) -> p w d", d=GC),
                        )
                    else:
                        nc.sync.dma_start(
                            out=x1shard[wlo * 128:(wlo + CW) * 128, :].rearrange(
                                "(w p) d -> p w d", p=128
                            ),
                            in_=x1_sb[:, wlo * D:(wlo + CW) * D].rearrange(
                                "p (w d) -> p w d", d=D
                            ),
                        )
                else:
                    nc.sync.dma_start(
                        out=out_d[wlo * 128:(wlo + CW) * 128, :].rearrange(
                            "(w p) d -> p w d", p=128
                        ),
                        in_=outc[:].rearrange("p (w d) -> p w d", d=3 * D),
                    )

        do_round(xg1, False)
        nc.gpsimd.collective_compute(
            "AllGather",
            mybir.AluOpType.bypass,
            replica_groups=[list(range(NCORES))],
            ins=[x1shard[:]],
            outs=[x1full[:]],
        )
        do_round(x1full, True)

    nc.finalize()
    return nc


def _make_in_maps(feat, src, dst, w_e):
    GL, GH, tables = _preprocess(src, dst, w_e)
    xpad = np.zeros((NPAD, D), np.float32)
    xpad[:N] = np.asarray(feat, np.float32)
    if BF16:
        import ml_dtypes
        xg = np.zeros((NPAD, 2 * D), ml_dtypes.bfloat16)
        xg[:, :D] = xpad.astype(ml_dtypes.bfloat16)
    else:
        xg = xpad
    in_maps = []
    for c in range(NCORES):
        t = tables[c]
        in_maps.append(
            {
                "xg1": xg,
                "x0own": xpad[c * ROWS:(c + 1) * ROWS],
                "idx16": t["idx16"],
                "dstl": t["dstl"],
                "wts": t["wts"],
            }
        )
    return GL, GH, in_maps


_CACHE = {}


def _get_program(feat, src, dst, lambda_max):
    re_norm = float(np.float32(2.0) / np.asarray(lambda_max, np.float32).reshape(-1)[0])
    deg = np.bincount(np.asarray(dst, np.int64), minlength=N).astype(np.float32)
    d_inv = np.maximum(deg, 1.0) ** -0.5
    src64 = np.asarray(src, np.int64)
    dst64 = np.asarray(dst, np.int64)
    w_e = (d_inv[src64] * d_inv[dst64]).astype(np.float32)
    GL, GH, in_maps = _make_in_maps(feat, src, dst, w_e)
    key = (GL, GH, re_norm)
    if key not in _CACHE:
        _CACHE[key] = _build_bass(GL, GH, re_norm)
    return _CACHE[key], in_maps


def kernel(feat, src, dst, lambda_max):
    from concourse.bass_utils import run_bass_kernel_spmd

    nc, in_maps = _get_program(feat, src, dst, lambda_max)
    res = run_bass_kernel_spmd(nc, in_maps, core_ids=list(range(NCORES)))
    kernel.last_exec_time_ns = res.exec_time_ns
    out = np.concatenate([res.results[c]["out"] for c in range(NCORES)], axis=0)
    return np.ascontiguousarray(out[:N])


# revision 23
# speedup vs baseline: 1.0342x; 1.0153x over previous
"""ChebConv (K=3) GNN message passing on 8 Trainium2 NeuronCores.

Strategy (1D node partition, per sharding hint):
  - Nodes padded to NPAD rows and split into 8 contiguous blocks; core c owns
    dst rows [c*ROWS, (c+1)*ROWS) = WPC windows of 128 dst nodes each.
  - Both D^-1/2 scalings are folded into per-edge weights
    w_e = dinv[src] * dinv[dst], so each unnormalized-Laplacian application is
    a pure weighted segment-sum: h[dst] = sum_e w_e * x[src_e].
  - On device, each window's edges are processed as tiles of 128 edge slots:
    dma_gather fetches x[src] rows ([128 slots, 64] per tile), one fused DVE
    tensor_scalar builds the weighted one-hot lhsT ([slot, dst_local] =
    w * (iota == dstl)), and the tensor engine accumulates the window's
    segment-sum in PSUM across the window's tiles.
  - Chebyshev recurrence combines are node-local (ACT/DVE); X1 shards are
    exchanged between rounds via an on-device AllGather.
  - Edge slots are split into low/high src halves (gather indices are int16)
    and padded to uniform per-window tile counts (GL low + GH high) so the
    same NEFF runs on all 8 cores; per-core variation lives in input tables.
"""

import sys

for _p in ("/opt/trn_rl_repo",):
    if _p not in sys.path:
        sys.path.insert(0, _p)

import numpy as np

# Problem shape (hardcoded per contract).
N, E, D = 60000, 1200000, 64
NCORES = 8
WIN = 128           # dst nodes per window (PSUM partition dim)
WPC = 60            # windows per core
ROWS = WPC * WIN    # 7680 dst rows per core
NPAD = NCORES * ROWS  # 61440 padded node rows
SPLIT = 32768       # int16 gather index range per source half
CW = 6              # windows per gather chunk (WPC % CW == 0)
BF16 = True         # bf16 gather rows + one-hot (4x DVE mode, PE FWL)


def _preprocess(src, dst, w_e):
    """Build per-core gather/one-hot tables.

    Returns (GL, GH, tables) where tables[c] = dict(idx16, dstl, wts).
    """
    src = np.asarray(src, dtype=np.int64)
    dst = np.asarray(dst, dtype=np.int64)
    w_e = np.asarray(w_e, dtype=np.float32)

    gwin = dst // WIN          # global window id
    core = gwin // WPC
    wloc = gwin % WPC
    low = src < SPLIT

    # Per (core, window, half) tile counts -> uniform GL / GH.
    nlo = np.zeros((NCORES, WPC), np.int64)
    nhi = np.zeros((NCORES, WPC), np.int64)
    np.add.at(nlo, (core[low], wloc[low]), 1)
    np.add.at(nhi, (core[~low], wloc[~low]), 1)
    GL = int(np.max(np.ceil(nlo / 128)))
    GH = int(np.max(np.ceil(nhi / 128)))
    G = GL + GH
    TILES = WPC * G
    SLOTS = TILES * 128
    HI0 = WPC * GL * 128  # high-stream slot offset

    # Sort edges by (core, window, high-flag), then by src within each run
    # (src-sorted gather streams improve HBM row locality).
    key = (core * WPC + wloc) * 2 + (~low)
    order = np.argsort(key * (1 << 17) + src, kind="stable")
    s_src = src[order]
    s_w = w_e[order]
    s_dstl = (dst[order] % WIN).astype(np.float32)
    s_low = low[order]

    # Start offset of each (core, window, half) run in the sorted arrays.
    counts = np.zeros(NCORES * WPC * 2, np.int64)
    np.add.at(counts, key, 1)
    starts = np.concatenate([[0], np.cumsum(counts)])

    tables = []
    for c in range(NCORES):
        idx_slot = np.zeros(SLOTS, np.int16)
        w_slot = np.zeros(SLOTS, np.float32)
        dstl_slot = np.zeros(SLOTS, np.float32)
        for w in range(WPC):
            kbase = (c * WPC + w) * 2
            # low half
            a, b = starts[kbase], starts[kbase + 1]
            n = b - a
            o = w * GL * 128
            if n:
                assert s_low[a:b].all()
                idx_slot[o:o + n] = s_src[a:b].astype(np.int16)
                w_slot[o:o + n] = s_w[a:b]
                dstl_slot[o:o + n] = s_dstl[a:b]
            # high half
            a, b = starts[kbase + 1], starts[kbase + 2]
            n = b - a
            o = HI0 + w * GH * 128
            if n:
                assert not s_low[a:b].any()
                idx_slot[o:o + n] = (s_src[a:b] - SPLIT).astype(np.int16)
                w_slot[o:o + n] = s_w[a:b]
                dstl_slot[o:o + n] = s_dstl[a:b]

        # idx16 layout: stream pos i -> [i % 16, i // 16], replicated to
        # all 8 groups of 16 partitions (one per GPSIMD Q7 core).
        idx16 = np.tile(idx_slot.reshape(SLOTS // 16, 16).T, (8, 1))
        # Per-tile tables in STREAM tile order: [partition(slot%128), tile].
        dstl = dstl_slot.reshape(TILES, 128).T.copy()
        wts = w_slot.reshape(TILES, 128).T.copy()
        tables.append({"idx16": idx16, "dstl": dstl, "wts": wts})

    return GL, GH, tables


def _build_bass(GL, GH, re_norm):
    import concourse.bass as bass
    import concourse.bacc as bacc
    import concourse.mybir as mybir
    import concourse.tile as tile
    from contextlib import ExitStack

    f32 = mybir.dt.float32
    i16 = mybir.dt.int16
    bf16 = mybir.dt.bfloat16
    gdt = bf16 if BF16 else f32      # gather-row / one-hot dtype
    GC = 2 * D if BF16 else D        # gather row width (256B either way)
    AF = mybir.ActivationFunctionType
    OP = mybir.AluOpType

    G = GL + GH
    TILES = WPC * G
    SLOTS = TILES * 128
    HI0 = WPC * GL * 128

    a1 = float(-re_norm)            # X1 = a1*h1 + b1*X0
    b1 = float(re_norm - 1.0)
    a2 = float(-2.0 * re_norm)      # X2 = a2*h2 + b2*X1 - X0
    b2 = float(2.0 * (re_norm - 1.0))

    nc = bacc.Bacc(
        "TRN2",
        target_bir_lowering=False,
        debug=False,
        enable_asserts=False,
        num_devices=NCORES,
        num_swdge_queues=2,
    )
    xg1 = nc.dram_tensor("xg1", [NPAD, GC], gdt, kind="ExternalInput")
    x0own = nc.dram_tensor("x0own", [ROWS, D], f32, kind="ExternalInput")
    idx16_d = nc.dram_tensor("idx16", [128, SLOTS // 16], i16, kind="ExternalInput")
    dstl_d = nc.dram_tensor("dstl", [128, TILES], f32, kind="ExternalInput")
    wts_d = nc.dram_tensor("wts", [128, TILES], f32, kind="ExternalInput")
    out_d = nc.dram_tensor("out", [ROWS, 3 * D], f32, kind="ExternalOutput")
    import ml_dtypes
    _iota_np = np.broadcast_to(np.arange(128), (128, 128))
    iota_d = nc.inline_tensor(
        _iota_np.astype(ml_dtypes.bfloat16 if BF16 else np.float32),
        name="iota",
    )

    with ExitStack() as ctx:
        tc = ctx.enter_context(tile.TileContext(nc))
        dram = ctx.enter_context(tc.tile_pool(name="dram", bufs=1, space="DRAM"))
        x1shard = dram.tile([ROWS, GC], gdt)
        x1full = dram.tile([NPAD, GC], gdt, addr_space="Shared")

        cpool = ctx.enter_context(tc.tile_pool(name="const", bufs=1))
        idx_sb = cpool.tile([128, SLOTS // 16], i16)
        nc.sync.dma_start(out=idx_sb[:], in_=idx16_d[:])
        dstl_sb = cpool.tile([128, TILES], f32)
        nc.sync.dma_start(out=dstl_sb[:], in_=dstl_d[:])
        wts_sb = cpool.tile([128, TILES], f32)
        nc.sync.dma_start(out=wts_sb[:], in_=wts_d[:])
        iota_sb = cpool.tile([128, 128], gdt)
        nc.sync.dma_start(out=iota_sb[:], in_=iota_d[:])
        x0_sb = cpool.tile([128, WPC * D], f32)
        nc.sync.dma_start(
            out=x0_sb[:].rearrange("p (w d) -> p w d", d=D),
            in_=x0own[:].rearrange("(w p) d -> p w d", p=128),
        )
        x1_sb = cpool.tile([128, WPC * D], f32)

        gpool = ctx.enter_context(tc.tile_pool(name="gath", bufs=2))
        ohpool = ctx.enter_context(tc.tile_pool(name="oh", bufs=8))
        pspool = ctx.enter_context(tc.tile_pool(name="ps", bufs=8, space="PSUM"))
        mpool = ctx.enter_context(tc.tile_pool(name="misc", bufs=4))
        opool = ctx.enter_context(tc.tile_pool(name="outs", bufs=3))

        L_low = CW * GL * 128
        L_high = CW * GH * 128
        reg_low = nc.gpsimd.alloc_register("n_idx_low")
        nc.gpsimd.reg_mov(reg_low, L_low)
        if L_high != L_low:
            reg_high = nc.gpsimd.alloc_register("n_idx_high")
            nc.gpsimd.reg_mov(reg_high, L_high)
        else:
            reg_high = reg_low

        def do_round(xsrc, second):
            for wlo in range(0, WPC, CW):
                glow = gpool.tile([128, CW * GL, GC], gdt, tag="glow")
                ghigh = gpool.tile([128, CW * GH, GC], gdt, tag="ghigh")
                s0 = wlo * GL * 128
                L = CW * GL * 128
                nc.gpsimd.dma_gather(
                    out_ap=glow[:, :, :],
                    in_ap=xsrc[0:SPLIT, :],
                    idxs_ap=idx_sb[:, s0 // 16:(s0 + L) // 16],
                    num_idxs=L,
                    num_idxs_reg=reg_low,
                    elem_size=GC,
                    single_packet=False,
                )
                s0h = HI0 + wlo * GH * 128
                Lh = CW * GH * 128
                nc.gpsimd.dma_gather(
                    out_ap=ghigh[:, :, :],
                    in_ap=xsrc[SPLIT:NPAD, :],
                    idxs_ap=idx_sb[:, s0h // 16:(s0h + Lh) // 16],
                    num_idxs=Lh,
                    num_idxs_reg=reg_high,
                    elem_size=GC,
                    single_packet=False,
                    queue_num=1,
                )
                if second:
                    outc = opool.tile([128, CW * 3 * D], f32, tag="outc")
                else:
                    x1bf = None
                    if BF16:
                        x1bf = opool.tile([128, CW * GC], gdt, tag="x1bf",
                                          name="x1bf")
                        nc.gpsimd.memset(x1bf[:], 0)
                for wi in range(CW):
                    w = wlo + wi
                    ps = pspool.tile([128, D], f32)
                    for t in range(G):
                        if t < GL:
                            g_ap = glow[:, wi * GL + t, 0:D]
                            sti = w * GL + t
                        else:
                            g_ap = ghigh[:, wi * GH + (t - GL), 0:D]
                            sti = WPC * GL + w * GH + (t - GL)
                        oh = ohpool.tile([128, 128], gdt, tag="oh")
                        nc.vector.tensor_scalar(
                            out=oh[:],
                            in0=iota_sb[:],
                            scalar1=dstl_sb[:, sti:sti + 1],
                            scalar2=wts_sb[:, sti:sti + 1],
                            op0=OP.is_equal,
                            op1=OP.mult,
                        )
                        nc.tensor.matmul(
                            ps[:],
                            lhsT=oh[:],
                            rhs=g_ap,
                            start=(t == 0),
                            stop=(t == G - 1),
                        )
                    x0w = x0_sb[:, w * D:(w + 1) * D]
                    x1w = x1_sb[:, w * D:(w + 1) * D]
                    if not second:
                        # X1 = a1*h + b1*X0
                        tmp = mpool.tile([128, D], f32, tag="t1")
                        nc.scalar.activation(tmp[:], ps[:], AF.Copy, scale=a1)
                        if b1 == 1.0:
                            nc.vector.tensor_tensor(
                                out=x1w, in0=tmp[:], in1=x0w, op=OP.add
                            )
                        else:
                            xb = mpool.tile([128, D], f32, tag="t2")
                            nc.scalar.activation(xb[:], x0w, AF.Copy, scale=b1)
                            nc.vector.tensor_tensor(
                                out=x1w, in0=tmp[:], in1=xb[:], op=OP.add
                            )
                        if BF16:
                            nc.vector.tensor_copy(
                                out=x1bf[:, wi * GC:wi * GC + D], in_=x1w
                            )
                    else:
                        # X2 = a2*h + b2*X1 - X0
                        tmp = mpool.tile([128, D], f32, tag="t1")
                        nc.scalar.activation(tmp[:], ps[:], AF.Copy, scale=a2)
                        xb = mpool.tile([128, D], f32, tag="t2")
                        nc.scalar.activation(xb[:], x1w, AF.Copy, scale=b2)
                        t3 = mpool.tile([128, D], f32, tag="t3")
                        nc.vector.tensor_tensor(
                            out=t3[:], in0=tmp[:], in1=xb[:], op=OP.add
                        )
                        x2 = mpool.tile([128, D], f32, tag="t4")
                        nc.vector.tensor_tensor(
                            out=x2[:], in0=t3[:], in1=x0w, op=OP.subtract
                        )
                        # relu into the chunk output staging tile
                        ob = wi * 3 * D
                        nc.scalar.activation(outc[:, ob:ob + D], x0w, AF.Relu)
                        nc.scalar.activation(outc[:, ob + D:ob + 2 * D], x1w, AF.Relu)
                        nc.scalar.activation(outc[:, ob + 2 * D:ob + 3 * D], x2[:], AF.Relu)
                if not second:
                    # ship this chunk's X1 windows to the DRAM shard
                    if BF16:
                        nc.sync.dma_start(
                            out=x1shard[wlo * 128:(wlo + CW) * 128, :].rearrange(
                                "(w p) d -> p w d", p=128
                            ),
                            in_=x1bf[:].rearrange("p (w d) -> p w d", d=GC),
                        )
                    else:
                        nc.sync.dma_start(
                            out=x1shard[wlo * 128:(wlo + CW) * 128, :].rearrange(
                                "(w p) d -> p w d", p=128
                            ),
                            in_=x1_sb[:, wlo * D:(wlo + CW) * D].rearrange(
                                "p (w d) -> p w d", d=D
                            ),
                        )
                else:
                    nc.sync.dma_start(
                        out=out_d[wlo * 128:(wlo + CW) * 128, :].rearrange(
                            "(w p) d -> p w d", p=128
                        ),
                        in_=outc[:].rearrange("p (w d) -> p w d", d=3 * D),
                    )

        do_round(xg1, False)
        nc.gpsimd.collective_compute(
            "AllGather",
            mybir.AluOpType.bypass,
            replica_groups=[list(range(NCORES))],
            ins=[x1shard[:]],
            outs=[x1full[:]],
        )
        do_round(x1full, True)

    nc.finalize()
    return nc


def _make_in_maps(feat, src, dst, w_e):
    GL, GH, tables = _preprocess(src, dst, w_e)
    xpad = np.zeros((NPAD, D), np.float32)
    xpad[:N] = np.asarray(feat, np.float32)
    if BF16:
        import ml_dtypes
        xg = np.zeros((NPAD, 2 * D), ml_dtypes.bfloat16)
        xg[:, :D] = xpad.astype(ml_dtypes.bfloat16)
    else:
        xg = xpad
    in_maps = []
    for c in range(NCORES):
        t = tables[c]
        in_maps.append(
            {
                "xg1": xg,
                "x0own": xpad[c * ROWS:(c + 1) * ROWS],
                "idx16": t["idx16"],
                "dstl": t["dstl"],
                "wts": t["wts"],
            }
        )
    return GL, GH, in_maps


_CACHE = {}


def _get_program(feat, src, dst, lambda_max):
    re_norm = float(np.float32(2.0) / np.asarray(lambda_max, np.float32).reshape(-1)[0])
    deg = np.bincount(np.asarray(dst, np.int64), minlength=N).astype(np.float32)
    d_inv = np.maximum(deg, 1.0) ** -0.5
    src64 = np.asarray(src, np.int64)
    dst64 = np.asarray(dst, np.int64)
    w_e = (d_inv[src64] * d_inv[dst64]).astype(np.float32)
    GL, GH, in_maps = _make_in_maps(feat, src, dst, w_e)
    key = (GL, GH, re_norm)
    if key not in _CACHE:
        _CACHE[key] = _build_bass(GL, GH, re_norm)
    return _CACHE[key], in_maps


def kernel(feat, src, dst, lambda_max):
    from concourse.bass_utils import run_bass_kernel_spmd

    nc, in_maps = _get_program(feat, src, dst, lambda_max)
    res = run_bass_kernel_spmd(nc, in_maps, core_ids=list(range(NCORES)))
    kernel.last_exec_time_ns = res.exec_time_ns
    out = np.concatenate([res.results[c]["out"] for c in range(NCORES)], axis=0)
    return np.ascontiguousarray(out[:N])
